# revision 27
# baseline (speedup 1.0000x reference)
"""Trainium2 Bass kernel: 3x3 VALID conv (NHWC) with weight thresholding + bias.

Full-input contract: kernel(x, weight, bias) -> out
  x:      (32, 56, 56, 256) fp32 NHWC
  weight: (256, 256, 3, 3)  fp32 OIHW, |w| < 0.01 -> 0
  bias:   (256,)            fp32
  out:    (32, 54, 54, 256) fp32 NHWC

Sharding: data-parallel over batch, 4 images per core on 8 cores.

Device-side formulation: 1D Winograd F(2,3) along H + implicit GEMM along W.
For each pair of output rows (one "th" tile), the 3 kh-taps are replaced by
4 transformed components:
  V[0] = x[2th]   - x[2th+2]
  V[1] = x[2th+1] + x[2th+2]
  V[2] = x[2th+2] - x[2th+1]
  V[3] = x[2th+1] - x[2th+3]
  M[a][co, th, ow] = sum_{kw, ci} U[a,kw][ci,co]^T V[a][ci, th, ow+kw]
  y[2th]   = M[0] + M[1] + M[2] + bias
  y[2th+1] = M[1] - M[2] - M[3] + bias
with U[a,kw] = sum_kh G[a,kh] w_thresh[:, :, kh, kw] precomputed on host
(G rows: [1,0,0], [.5,.5,.5], [.5,-.5,.5], [0,0,1]; exact math).

This cuts PE matmul columns by 1.5x vs direct conv (24 passes per 2 rows
instead of 36): per core 576 matmuls x 486 cols = 280k PE cycles (117 us)
vs 420k direct. x/V/U are bf16 (matmul still 1 col/cycle, PSUM fp32;
conv rel err ~3e-3 vs the 2e-2 gate) which halves x DMA and lets the
V-transform run in DVE's 2x bf16 fast path (tensor_tensor, packed SBUF).
The output transform is spread across engines with at most one PSUM
operand per instruction (a HW constraint; GpSimd cannot touch PSUM):
  Act:    u = M1 + bias ,  s = copy(M2)     (PSUM -> SBUF)
  GpSimd: v = u + s     ,  w = u - s        (SBUF only)
  DVE:    y_even = v + M0 , y_odd = w - M3  (one PSUM read each)
x loads split across the SP and Pool DMA queues; weights via the Act
queue in per-(o,a) pieces so the first matmul starts at ~3.8 us. All
engine busy: PE 118 us, DVE 62, SP 61, Act 30, Pool 29; sim 126 us.

Host-side marshaling (part of sharding): x per core pre-transposed to
[256, 4*3136] channel-major bf16, U packed into bf16 [128, 48*128] lhsT
tiles, per-core output [256, 4*2916] fp32 transposed back to NHWC.
"""

import numpy as np
import ml_dtypes
from contextlib import ExitStack

import concourse.bass as bass
import concourse.bacc as bacc
import concourse.tile as tile
import concourse.mybir as mybir
from concourse.bass_utils import run_bass_kernel_spmd

N_CORES = 8
IMGS_PER_CORE = 4
H, W, C = 56, 56, 256
OH, OW, CO = 54, 54, 256
NPIX_IN = H * W       # 3136
NPIX_OUT = OH * OW    # 2916
P = 128
TH = 27               # 2-output-row tiles per image
BLK_TH = 9            # th tiles per PSUM block: 9*54 = 486 <= 512
N_BLKS = TH // BLK_TH  # 3
BLK = BLK_TH * OW      # 486
OT_COLS = 2 * BLK      # 972 = 18 output rows
SPARSE_TH = 0.01

TRACE = False
LAST = None  # BassKernelResults of the most recent run (for test harness)

_NC_CACHE = None


def _build_module(repeat=1, dummy_io=False):
    f32 = mybir.dt.float32
    bf16 = mybir.dt.bfloat16
    A = mybir.AluOpType

    nc = bacc.Bacc(
        "TRN2",
        target_bir_lowering=False,
        debug=False,
        enable_asserts=False,
        num_devices=N_CORES,
    )
    # dummy_io: identical instruction stream + DMA traffic, but all big
    # tensors Internal so the bench pays no per-call host<->device transfer.
    kin = "Internal" if dummy_io else "ExternalInput"
    kout = "Internal" if dummy_io else "ExternalOutput"
    xt = nc.dram_tensor("xt", [P, IMGS_PER_CORE * 2 * NPIX_IN], bf16, kind=kin).ap()
    wu = nc.dram_tensor("wu", [P, 48 * P], bf16, kind=kin).ap()
    b2 = nc.dram_tensor("b2", [P, 2], f32, kind=kin).ap()
    yt = nc.dram_tensor("yt", [CO, IMGS_PER_CORE * NPIX_OUT], f32, kind=kout).ap()
    if dummy_io:
        done = nc.dram_tensor("done", [1, 1], f32, kind="ExternalOutput").ap()

    with tile.TileContext(nc) as tc, ExitStack() as ctx:
        wpool = ctx.enter_context(tc.tile_pool(name="w", bufs=1))
        bpool = ctx.enter_context(tc.tile_pool(name="b", bufs=1))
        xpool = ctx.enter_context(tc.tile_pool(name="x", bufs=2))
        vpool = ctx.enter_context(tc.tile_pool(name="v", bufs=4))
        opool = ctx.enter_context(tc.tile_pool(name="o", bufs=4))
        tpool = ctx.enter_context(tc.tile_pool(name="t", bufs=2))
        pspool = ctx.enter_context(tc.tile_pool(name="ps", bufs=8, space="PSUM"))

        w_sb = wpool.tile([P, 48 * P], bf16)
        b_sb = bpool.tile([P, 2], f32)

        QROWS = 14
        QUART = QROWS * W  # 14 input rows = 784 pixels

        def load_x_quarter(xc_tile, img, c, q):
            # c0 quarters issue from SP, c1 from Pool: two DMA queues in
            # parallel, halving the x-load critical path.
            base = img * 2 * NPIX_IN + c * NPIX_IN
            eng = nc.sync if c == 0 else nc.gpsimd
            eng.dma_start(
                xc_tile[:, c * NPIX_IN + q * QUART: c * NPIX_IN + (q + 1) * QUART],
                xt[:, base + q * QUART: base + (q + 1) * QUART],
            )

        def emit_w_piece(t6):
            nc.scalar.dma_start(w_sb[:, t6 * 6 * P:(t6 + 1) * 6 * P],
                                wu[:, t6 * 6 * P:(t6 + 1) * 6 * P])

        stt = nc.vector.scalar_tensor_tensor

        for rep, img in [(r, i) for r in range(repeat) for i in range(IMGS_PER_CORE)]:
            first = (rep == 0 and img == 0)
            xc = xpool.tile([P, 2 * NPIX_IN], bf16, tag="x", name=f"x_{rep}_{img}")
            # V transform on DVE: per ci chunk, [128, 4 comps, 27 th, 56 w],
            # emitted in block-aligned th pieces interleaved with the x
            # quarter DMAs each piece needs, so nothing over-waits.
            vs = []
            xs2s = []
            for c in range(2):
                vt = vpool.tile([P, 4 * TH * W], bf16, tag="v",
                                name=f"v_{rep}_{img}_{c}")
                xs2s.append(xc[:, c * NPIX_IN:(c + 1) * NPIX_IN].rearrange(
                    "p (th two w) -> p th two w", two=2, w=W))
                vs.append(vt[:].rearrange("p (a th w) -> p a th w", a=4, w=W))

            def emit_v_piece(t0, t1):
                for a in range(4):
                    for c in range(2):
                        xs2, v4 = xs2s[c], vs[c]
                        r0 = xs2[:, t0:t1, 0, :]          # x[2th]
                        r1 = xs2[:, t0:t1, 1, :]          # x[2th+1]
                        r2 = xs2[:, t0 + 1:t1 + 1, 0, :]  # x[2th+2]
                        r3 = xs2[:, t0 + 1:t1 + 1, 1, :]  # x[2th+3]
                        if a == 0:
                            nc.vector.tensor_tensor(v4[:, 0, t0:t1, :], r0, r2, A.subtract)
                        elif a == 1:
                            nc.vector.tensor_tensor(v4[:, 1, t0:t1, :], r1, r2, A.add)
                        elif a == 2:
                            nc.vector.tensor_tensor(v4[:, 2, t0:t1, :], r2, r1, A.subtract)
                        else:
                            nc.vector.tensor_tensor(v4[:, 3, t0:t1, :], r1, r3, A.subtract)

            for q in range(2):
                for c in range(2):
                    load_x_quarter(xc, img, c, q)
            emit_v_piece(0, 9)
            if first:
                for t6 in range(4):   # o=0, a=0..3
                    emit_w_piece(t6)
                nc.sync.dma_start(b_sb[:], b2)
            for q in range(2, 4):
                for c in range(2):
                    load_x_quarter(xc, img, c, q)
            if first:
                for t6 in range(4, 8):  # o=1, a=0..3
                    emit_w_piece(t6)
            emit_v_piece(9, 18)
            emit_v_piece(18, TH)

            for blk in range(N_BLKS):
                th0 = blk * BLK_TH
                for o in range(2):
                    p3 = []
                    for a in range(4):
                        ps = pspool.tile([P, BLK], f32, tag="ps",
                                         name=f"ps_{rep}_{img}_{blk}_{o}_{a}")
                        mm = 0
                        for kw in range(3):
                            for c in range(2):
                                t = ((o * 4 + a) * 3 + kw) * 2 + c
                                rhs = vs[c][:, a, th0:th0 + BLK_TH, kw:kw + OW]
                                nc.tensor.matmul(
                                    ps[:], w_sb[:, t * P:(t + 1) * P], rhs,
                                    start=(mm == 0), stop=(mm == 5),
                                )
                                mm += 1
                        p3.append(ps[:].rearrange("p (th w) -> p th w", w=OW))

                    ot = opool.tile([P, OT_COLS], f32, tag="ot",
                                    name=f"ot_{rep}_{img}_{blk}_{o}")
                    o3 = ot[:].rearrange("p (th two w) -> p th two w", two=2, w=OW)
                    bsc = b_sb[:, o:o + 1]
                    # Each DVE op may read at most ONE PSUM operand; chain
                    # through SBUF temps, sharing u = ps1 + bias:
                    #   y_even = ps0 + (u + ps2) ; y_odd = (u - ps2) - ps3
                    tu = tpool.tile([P, BLK], f32, tag="tu", name=f"tu_{rep}_{img}_{blk}_{o}")
                    tv = tpool.tile([P, BLK], f32, tag="tv", name=f"tv_{rep}_{img}_{blk}_{o}")
                    tw = tpool.tile([P, BLK], f32, tag="tw", name=f"tw_{rep}_{img}_{blk}_{o}")
                    u3 = tu[:].rearrange("p (th w) -> p th w", w=OW)
                    v3 = tv[:].rearrange("p (th w) -> p th w", w=OW)
                    w3 = tw[:].rearrange("p (th w) -> p th w", w=OW)
                    # spread engines (GpSimd can't touch PSUM): Act pulls
                    # ps1+bias and ps2 into SBUF; DVE combines (u±s run in
                    # the 2x SBUF-only fast path), then the two PSUM adds.
                    ts = tpool.tile([P, BLK], f32, tag="ts", name=f"ts_{rep}_{img}_{blk}_{o}")
                    s3 = ts[:].rearrange("p (th w) -> p th w", w=OW)
                    col0 = img * NPIX_OUT + blk * OT_COLS
                    # For the very last group, split the drain into two th
                    # halves so the first DMA overlaps the second chain.
                    last = (rep == repeat - 1 and img == IMGS_PER_CORE - 1
                            and blk == N_BLKS - 1 and o == 1)
                    for s0, s1 in (((0, 5), (5, BLK_TH)) if last else ((0, BLK_TH),)):
                        nc.scalar.add(u3[:, s0:s1, :], p3[1][:, s0:s1, :], bsc)
                        nc.scalar.copy(s3[:, s0:s1, :], p3[2][:, s0:s1, :])
                        nc.gpsimd.tensor_tensor(v3[:, s0:s1, :], u3[:, s0:s1, :],
                                                s3[:, s0:s1, :], A.add)
                        stt(o3[:, s0:s1, 0, :], v3[:, s0:s1, :], 1.0,
                            p3[0][:, s0:s1, :], A.mult, A.add)
                        nc.gpsimd.tensor_tensor(w3[:, s0:s1, :], u3[:, s0:s1, :],
                                                s3[:, s0:s1, :], A.subtract)
                        stt(o3[:, s0:s1, 1, :], w3[:, s0:s1, :], 1.0,
                            p3[3][:, s0:s1, :], A.mult, A.subtract)
                        nc.sync.dma_start(
                            yt[o * P:(o + 1) * P,
                               col0 + s0 * 2 * OW: col0 + s1 * 2 * OW],
                            ot[:, s0 * 2 * OW: s1 * 2 * OW])
        if dummy_io:
            dpool = ctx.enter_context(tc.tile_pool(name="d", bufs=1))
            dt = dpool.tile([1, 1], f32)
            nc.scalar.memzero(dt[:])
            nc.sync.dma_start(done, dt[:])
    nc.compile()
    return nc


def _pack_inputs(x, weight, bias):
    """Host marshaling: threshold + H-Winograd-transform weights, shard x."""
    x = np.ascontiguousarray(np.asarray(x, dtype=np.float32))
    weight = np.asarray(weight, dtype=np.float32)
    bias = np.asarray(bias, dtype=np.float32)

    w = np.where(np.abs(weight) < SPARSE_TH, np.float32(0.0), weight)
    G = np.array([[1, 0, 0], [.5, .5, .5], [.5, -.5, .5], [0, 0, 1]], np.float32)
    # U[a, kw, co, ci] = sum_kh G[a, kh] * w[co, ci, kh, kw]
    U = np.einsum("ak,oikw->awoi", G, w).astype(np.float32)
    # lhsT tile for t=((o*4+a)*3+kw)*2+c is U[a,kw][ci chunk c, co chunk o].T
    wu = np.empty((P, 48 * P), np.float32)
    for o in range(2):
        for a in range(4):
            for kw in range(3):
                for c in range(2):
                    t = ((o * 4 + a) * 3 + kw) * 2 + c
                    blkw = U[a, kw, o * P:(o + 1) * P, c * P:(c + 1) * P]  # [co,ci]
                    wu[:, t * P:(t + 1) * P] = blkw.T
    wu = wu.astype(ml_dtypes.bfloat16)
    b2 = np.ascontiguousarray(bias.reshape(2, P).T)  # [co_in, co_chunk]

    xb = x.astype(ml_dtypes.bfloat16)
    in_maps = []
    for i in range(N_CORES):
        xc = xb[i * IMGS_PER_CORE:(i + 1) * IMGS_PER_CORE]      # [4,56,56,256]
        xt_i = np.ascontiguousarray(
            xc.reshape(IMGS_PER_CORE, NPIX_IN, 2, P)
            .transpose(3, 0, 2, 1)
            .reshape(P, IMGS_PER_CORE * 2 * NPIX_IN)
        )
        in_maps.append({"xt": xt_i, "wu": wu, "b2": b2})
    return in_maps


def make_in_maps(x, weight, bias):
    return _pack_inputs(x, weight, bias)


def kernel(x, weight, bias):
    global _NC_CACHE, LAST
    in_maps = _pack_inputs(x, weight, bias)

    if _NC_CACHE is None:
        _NC_CACHE = _build_module()
    nc = _NC_CACHE

    LAST = run_bass_kernel_spmd(
        nc, in_maps, core_ids=list(range(N_CORES)), trace=TRACE
    )

    out = np.empty((32, OH, OW, CO), np.float32)
    for i in range(N_CORES):
        ytc = LAST.results[i]["yt"]                 # [256, 4*2916]
        out[i * IMGS_PER_CORE:(i + 1) * IMGS_PER_CORE] = (
            ytc.reshape(CO, IMGS_PER_CORE, OH, OW).transpose(1, 2, 3, 0)
        )
    return out
